# revision 32
# baseline (speedup 1.0000x reference)
"""BiLinearInteraction Trainium2 kernel (8 NeuronCores, data-parallel over batch).

Reference computation (per pair p=(i,j) of F=26 fields, P=325 pairs):
    out[b, p*64:(p+1)*64] = (x[i, b, :] @ W[p]) * x[j, b, :]
Full shapes: x [26, 4096, 64] f32, W [325, 64, 64] f32 -> out [4096, 20800] f32.

Strategy
- Shard batch axis 4096 -> 8 x 512, replicate W (sharding hint).
- Host pre-packs per-core operands so the device kernel is a pure stream of
  bf16 matmuls + elementwise muls + large contiguous DMAs:
    xn  bf16 [512, 26*64]   natural layout (elementwise xj operand)
    xt  bf16 [128, 4*13*128] d-major (matmul lhsT): field i lives in SBUF
                            partition group (i%2)*64 only - each field's
                            matmul runs in one 64-row half of the PE array
                            (tile_position row tiling), so the two groups
                            carry disjoint fields and nothing is loaded twice.
    w   bf16 [128, 11008]   pair-grouped (matmul rhs), same even/odd field
                            split across partition groups.
- PE warm-up burst: the TRN2 PE clock is gated at 1.2 GHz until the HAM
  activity monitor sees ~3.4us of sustained matmul traffic, then 2.4 GHz.
  The matmul stream is this kernel's longest engine queue, so 16 dummy
  matmuls run back-to-back during the initial DMA load phase (when the PE
  would be idle anyway) to lift the clock before real work starts.
- Per batch tile (4 tiles of 128 rows) and left field i (pairs (i, i+1..25)
  are contiguous): matmul psum[128, n_i*64] = xt_i.T @ w cols, then the
  PSUM f32 product is combined with xn via one of three paths chosen by a
  greedy balance of measured engine rates (DVE 2x packed mode needs
  all-bf16 operands; PSUM reads force 1x; GPSIMD has no PSUM port):
    A: ScalarE drains PSUM (f32->bf16), DVE muls all-bf16 at 2x (~205 G/s)
    B: ScalarE drains, GpSimd muls bf16 (~59 G/s, otherwise idle)
    C: DVE muls straight from PSUM at 1x (~102 G/s)
  Chunks of whole fields are DMAed out as large contiguous transfers on the
  SP HWDGE ring while input loads ride SWDGE, keeping the write stream (the
  critical path: 21.3MB/core bf16) unblocked.
- Output staged and written as bf16 (halves the write stream); host upcasts.
"""

import sys

sys.path.insert(0, "/opt/trn_rl_repo")

from itertools import combinations

import ml_dtypes
import numpy as np

import concourse.bass as bass
import concourse.mybir as mybir
from concourse import bacc
from concourse.tile import TileContext

F, D, B = 26, 64, 4096
NCORES = 8
BC = B // NCORES          # 512 batch rows per core
NT = BC // 128            # 4 batch tiles of 128 rows
PAIRS = list(combinations(range(F), 2))
P = len(PAIRS)            # 325
OUT_COLS = P * D          # 20800
NSLOT = (F + 1) // 2      # 13 fields per partition group
N_WARM = 16               # PE warm-up matmuls (~6.8us at the cold clock)

# Fields grouped into output chunks (pair counts 25,24,...,1). Whole-field
# chunks keep both the staging tile and the xj slice contiguous.
CHUNKS = [range(0, 2), range(2, 4), range(4, 6), range(6, 10),
          range(10, 14), range(14, 19), range(19, 25)]
N_PAIRS = [F - 1 - i for i in range(F - 1)]          # pairs with left field i
P_START = [sum(N_PAIRS[:i]) for i in range(F - 1)]   # first pair index of field i

# PSUM is 8 banks of 512 f32 cols; pieces of <=1024 cols (2 banks) allow a
# 4-deep psum pool so the PE runs ahead of the drain stage instead of
# stalling (stalls re-throttle the HAM clock gate back to 1.2 GHz).
PIECE = 1024

def _pieces(cols):
    res = []
    s0 = 0
    while s0 < cols:
        res.append((s0, min(PIECE, cols - s0)))
        s0 += PIECE
    return res

# Greedy global balance of the elementwise stage over DVE / ACT / GPSIMD,
# per psum piece. Rates in ns per psum column of 128 rows (fit from HW
# traces, in-situ - they include SBUF-port contention with the DMA write
# stream), plus fixed per-instruction dispatch costs. Initial offsets:
# ACT's table load, GPSIMD's SWDGE descriptor generation for input loads.
def _assign_paths():
    tot = {"dve": 0.0, "act": 0.0, "gps": 0.0}
    path = {}
    for t in range(NT):
        for ch in CHUNKS:
            for i in ch:
                for s0, pc in _pieces(N_PAIRS[i] * D):
                    cand = []
                    # A: ACT drain + DVE 2x mul
                    cand.append(("A", {"act": pc * 0.90 + 293,
                                       "dve": pc * 0.624 + 150}))
                    # B: ACT drain + GPSIMD mul (large pieces only)
                    if pc >= 512:
                        cand.append(("B", {"act": pc * 0.90 + 293,
                                           "gps": pc * 1.75 + 400}))
                    # C: DVE 1x mul from PSUM
                    cand.append(("C", {"dve": pc * 1.30 + 150}))
                    best = None
                    for name, costs in cand:
                        trial = dict(tot)
                        for k, v in costs.items():
                            trial[k] += v
                        m = max(trial.values())
                        if best is None or m < best[0]:
                            best = (m, name, trial)
                    path[(t, i, s0)] = best[1]
                    tot = best[2]
    return path

PATH = _assign_paths()

# w SBUF/DRAM packing: per chunk, even fields pack into partitions 0-63,
# odd fields into 64-127, each group's pair-columns concatenated from col 0.
# Chunk width = max of the two groups' widths.
W_OFF = {}        # field -> col offset inside its chunk tile
W_CHUNK_W = []    # chunk -> tile width (cols)
for ch in CHUNKS:
    off = [0, 0]
    for i in ch:
        W_OFF[i] = off[i % 2]
        off[i % 2] += N_PAIRS[i] * D
    W_CHUNK_W.append(max(off))
W_COLS = sum(W_CHUNK_W)
W_CSTART = [sum(W_CHUNK_W[:ci]) for ci in range(len(CHUNKS))]

F32 = mybir.dt.float32
BF16 = mybir.dt.bfloat16


def build_bass() -> bass.Bass:
    # Bacc (not Bass): its compile() splits multi-sem waits into event
    # semaphores - TRN2 engine instructions take at most one inline wait.
    nc = bacc.Bacc()
    xn = nc.declare_dram_parameter("xn", [BC, F * D], BF16, isOutput=False)
    xt = nc.declare_dram_parameter("xt", [2 * D, NT * NSLOT * 128], BF16,
                                   isOutput=False)
    w = nc.declare_dram_parameter("w", [2 * D, W_COLS], BF16, isOutput=False)
    out = nc.declare_dram_parameter("out", [BC, OUT_COLS], BF16, isOutput=True)

    with TileContext(nc) as tc:
        with (
            tc.tile_pool(name="consts", bufs=1) as consts,
            tc.tile_pool(name="xn_pool", bufs=4) as xn_pool,
            tc.tile_pool(name="stage", bufs=7) as stage_pool,
            tc.tile_pool(name="cp_pool", bufs=6) as cp_pool,
            tc.tile_pool(name="psum", bufs=4, space="PSUM") as psum_pool,
        ):
            # PE warm-up: zeroed operands, results never read. Emitted first
            # so the matmul queue streams during the input-load phase.
            warm_w = consts.tile([D, 128], BF16, tag="warm_w", name="warm_w")
            warm_x = consts.tile([D, 512], BF16, tag="warm_x", name="warm_x")
            nc.vector.memset(warm_w[:], 0.0)
            nc.vector.memset(warm_x[:], 0.0)
            for _ in range(N_WARM):
                wps = psum_pool.tile([128, 512], F32, tag="ps")
                nc.tensor.matmul(wps[:], warm_w[:], warm_x[:],
                                 start=True, stop=True)

            w_sb = [consts.tile([2 * D, cw], BF16, tag=f"w{ci}", name=f"w{ci}")
                    for ci, cw in enumerate(W_CHUNK_W)]
            xt_sb = [consts.tile([2 * D, NSLOT * 128], BF16, tag=f"xt{t}",
                                 name=f"xtsb{t}")
                     for t in range(NT)]
            xn_sb = [xn_pool.tile([128, F * D], BF16, tag="xn", name=f"xn{t}")
                     for t in range(NT)]

            # Input loads, issued up front and ordered by first use. The
            # first ~1.7MB rides the HWDGE (nc.sync) ring: SWDGE descriptor
            # generation is dead until the GPSIMD ucode load finishes (~6us
            # of MODIFY_POOL_CONFIG at kernel start), while HWDGE needs no
            # Q7 software and streams from t~0. The rest rides SWDGE
            # (nc.gpsimd) so it never queues behind the output write stream
            # (HWDGE is strict FIFO per ring).
            def _load_w(ci, eng):
                eng.dma_start(
                    out=w_sb[ci][:],
                    in_=w[:, W_CSTART[ci]:W_CSTART[ci] + W_CHUNK_W[ci]])

            def _load_t(t, eng):
                s = t * NSLOT * 128
                eng.dma_start(out=xt_sb[t][:], in_=xt[:, s:s + NSLOT * 128])
                eng.dma_start(out=xn_sb[t][:],
                              in_=xn[t * 128:(t + 1) * 128, :])

            _load_w(0, nc.gpsimd); _load_t(0, nc.gpsimd)
            _load_w(1, nc.gpsimd); _load_w(2, nc.gpsimd)
            _load_t(1, nc.gpsimd); _load_w(3, nc.gpsimd); _load_w(4, nc.gpsimd)
            _load_t(2, nc.gpsimd); _load_w(5, nc.gpsimd); _load_w(6, nc.gpsimd)
            _load_t(3, nc.gpsimd)

            for t in range(NT):
                for ci, ch in enumerate(CHUNKS):
                    ccol0 = P_START[ch[0]] * D
                    ccol1 = (P_START[ch[-1]] + N_PAIRS[ch[-1]]) * D
                    ccols = ccol1 - ccol0
                    st = stage_pool.tile([128, ccols], BF16, tag="stage")
                    for i in ch:
                        npair = N_PAIRS[i]
                        cols = npair * D
                        wcol0 = P_START[i] * D
                        r0 = (i % 2) * D  # PE row group alternates per field
                        slot = i // 2
                        lhsT = xt_sb[t][r0:r0 + D, slot * 128:(slot + 1) * 128]
                        woff = W_OFF[i]
                        for s0, pc in _pieces(cols):
                            ps = psum_pool.tile([128, pc], F32, tag="ps")
                            for q0 in range(0, pc, 512):
                                n = min(512, pc - q0)
                                c0 = woff + s0 + q0
                                nc.tensor.matmul(
                                    ps[:, q0:q0 + n], lhsT,
                                    w_sb[ci][r0:r0 + D, c0:c0 + n],
                                    start=True, stop=True,
                                )
                            d0 = wcol0 - ccol0 + s0
                            dst = st[:, d0:d0 + pc]
                            xj = xn_sb[t][:, (i + 1) * D + s0:
                                          (i + 1) * D + s0 + pc]
                            p = PATH[(t, i, s0)]
                            if p in ("A", "B"):
                                cp = cp_pool.tile([128, pc], BF16, tag="cp")
                                nc.scalar.copy(out=cp[:], in_=ps[:])
                                eng = nc.vector if p == "A" else nc.gpsimd
                                eng.tensor_mul(dst, cp[:], xj)
                            else:
                                nc.vector.tensor_mul(dst, ps[:], xj)
                    nc.sync.dma_start(
                        out=out[t * 128:(t + 1) * 128, ccol0:ccol0 + ccols],
                        in_=st[:],
                    )
    nc.compile()
    return nc


def prep_inputs(x: np.ndarray, W: np.ndarray):
    """Full inputs -> per-core in_maps with pre-packed layouts."""
    x = np.ascontiguousarray(np.asarray(x, dtype=np.float32))
    W = np.ascontiguousarray(np.asarray(W, dtype=np.float32))
    # w: [128, W_COLS] bf16; chunk ci at cols W_CSTART[ci]; field i in rows
    # (i%2)*64..+64 at chunk-local col W_OFF[i]; identical on every core.
    wg = np.zeros((2 * D, W_COLS), dtype=ml_dtypes.bfloat16)
    wt = W.transpose(1, 0, 2)  # [D, P, D]
    for ci, ch in enumerate(CHUNKS):
        for i in ch:
            r0 = (i % 2) * D
            c0 = W_CSTART[ci] + W_OFF[i]
            cols = N_PAIRS[i] * D
            wg[r0:r0 + D, c0:c0 + cols] = (
                wt[:, P_START[i]:P_START[i] + N_PAIRS[i], :]
                .reshape(D, cols).astype(ml_dtypes.bfloat16)
            )
    in_maps = []
    for c in range(NCORES):
        xc = x[:, c * BC:(c + 1) * BC, :]                      # [26, 512, 64]
        xn = np.ascontiguousarray(
            xc.transpose(1, 0, 2).reshape(BC, F * D).astype(ml_dtypes.bfloat16)
        )
        # xt: [128, NT*13*128]; tile t, field i -> rows (i%2)*64..+64,
        # cols (t*13 + i//2)*128..+128, content xc[i, trows, :].T
        xt = np.zeros((2 * D, NT * NSLOT * 128), dtype=ml_dtypes.bfloat16)
        xct = (xc.reshape(F, NT, 128, D).transpose(0, 1, 3, 2)
               .astype(ml_dtypes.bfloat16))                    # [F, NT, D, 128]
        for i in range(F):
            r0 = (i % 2) * D
            for t in range(NT):
                c0 = (t * NSLOT + i // 2) * 128
                xt[r0:r0 + D, c0:c0 + 128] = xct[i, t]
        in_maps.append({"xn": xn, "xt": np.ascontiguousarray(xt), "w": wg})
    return in_maps


_CACHED_NC = None


def kernel(x: np.ndarray, W: np.ndarray) -> np.ndarray:
    global _CACHED_NC
    from concourse.bass_utils import run_bass_kernel_spmd

    if _CACHED_NC is None:
        _CACHED_NC = build_bass()
    in_maps = prep_inputs(x, W)
    res = run_bass_kernel_spmd(_CACHED_NC, in_maps, list(range(NCORES)))
    shards = [
        np.asarray(res.results[c]["out"]).astype(np.float32) for c in range(NCORES)
    ]
    return np.concatenate(shards, axis=0)


# revision 33
# speedup vs baseline: 1.0123x; 1.0123x over previous
"""BiLinearInteraction Trainium2 kernel (8 NeuronCores, data-parallel over batch).

Reference computation (per pair p=(i,j) of F=26 fields, P=325 pairs):
    out[b, p*64:(p+1)*64] = (x[i, b, :] @ W[p]) * x[j, b, :]
Full shapes: x [26, 4096, 64] f32, W [325, 64, 64] f32 -> out [4096, 20800] f32.

Strategy
- Shard batch axis 4096 -> 8 x 512, replicate W (sharding hint).
- Host pre-packs per-core operands so the device kernel is a pure stream of
  bf16 matmuls + elementwise muls + large contiguous DMAs:
    xn  bf16 [512, 26*64]   natural layout (elementwise xj operand)
    xt  bf16 [128, 4*13*128] d-major (matmul lhsT): field i lives in SBUF
                            partition group (i%2)*64 only - each field's
                            matmul runs in one 64-row half of the PE array
                            (tile_position row tiling), so the two groups
                            carry disjoint fields and nothing is loaded twice.
    w   bf16 [128, 11008]   pair-grouped (matmul rhs), same even/odd field
                            split across partition groups.
- PE warm-up burst: the TRN2 PE clock is gated at 1.2 GHz until the HAM
  activity monitor sees ~3.4us of sustained matmul traffic, then 2.4 GHz.
  The matmul stream is this kernel's longest engine queue, so 16 dummy
  matmuls run back-to-back during the initial DMA load phase (when the PE
  would be idle anyway) to lift the clock before real work starts.
- Per batch tile (4 tiles of 128 rows) and left field i (pairs (i, i+1..25)
  are contiguous): matmul psum[128, n_i*64] = xt_i.T @ w cols, then the
  PSUM f32 product is combined with xn via one of three paths chosen by a
  greedy balance of measured engine rates (DVE 2x packed mode needs
  all-bf16 operands; PSUM reads force 1x; GPSIMD has no PSUM port):
    A: ScalarE drains PSUM (f32->bf16), DVE muls all-bf16 at 2x (~205 G/s)
    B: ScalarE drains, GpSimd muls bf16 (~59 G/s, otherwise idle)
    C: DVE muls straight from PSUM at 1x (~102 G/s)
  Chunks of whole fields are DMAed out as large contiguous transfers on the
  SP HWDGE ring while input loads ride SWDGE, keeping the write stream (the
  critical path: 21.3MB/core bf16) unblocked.
- Output staged and written as bf16 (halves the write stream); host upcasts.
"""

import sys

sys.path.insert(0, "/opt/trn_rl_repo")

from itertools import combinations

import ml_dtypes
import numpy as np

import concourse.bass as bass
import concourse.mybir as mybir
from concourse import bacc
from concourse.tile import TileContext

F, D, B = 26, 64, 4096
NCORES = 8
BC = B // NCORES          # 512 batch rows per core
NT = BC // 128            # 4 batch tiles of 128 rows
PAIRS = list(combinations(range(F), 2))
P = len(PAIRS)            # 325
OUT_COLS = P * D          # 20800
NSLOT = (F + 1) // 2      # 13 fields per partition group
N_WARM = 16               # PE warm-up matmuls (~6.8us at the cold clock)

# Fields grouped into output chunks (pair counts 25,24,...,1). Whole-field
# chunks keep both the staging tile and the xj slice contiguous.
CHUNKS = [range(0, 2), range(2, 4), range(4, 6), range(6, 10),
          range(10, 14), range(14, 19), range(19, 25)]
N_PAIRS = [F - 1 - i for i in range(F - 1)]          # pairs with left field i
P_START = [sum(N_PAIRS[:i]) for i in range(F - 1)]   # first pair index of field i

# PSUM is 8 banks of 512 f32 cols; pieces of <=1024 cols (2 banks) allow a
# 4-deep psum pool so the PE runs ahead of the drain stage instead of
# stalling (stalls re-throttle the HAM clock gate back to 1.2 GHz).
PIECE = 1024

def _pieces(cols):
    res = []
    s0 = 0
    while s0 < cols:
        res.append((s0, min(PIECE, cols - s0)))
        s0 += PIECE
    return res

# Greedy global balance of the elementwise stage over DVE / ACT / GPSIMD,
# per psum piece. Rates in ns per psum column of 128 rows (fit from HW
# traces, in-situ - they include SBUF-port contention with the DMA write
# stream), plus fixed per-instruction dispatch costs. Initial offsets:
# ACT's table load, GPSIMD's SWDGE descriptor generation for input loads.
def _assign_paths():
    tot = {"dve": 0.0, "act": 0.0, "gps": 0.0}
    path = {}
    for t in range(NT):
        for ch in CHUNKS:
            for i in ch:
                for s0, pc in _pieces(N_PAIRS[i] * D):
                    cand = []
                    # A: ACT drain + DVE 2x mul
                    cand.append(("A", {"act": pc * 0.90 + 293,
                                       "dve": pc * 0.624 + 150}))
                    # B: ACT drain + GPSIMD mul (large pieces only)
                    if pc >= 512:
                        cand.append(("B", {"act": pc * 0.90 + 293,
                                           "gps": pc * 1.75 + 400}))
                    # C: DVE 1x mul from PSUM
                    cand.append(("C", {"dve": pc * 1.30 + 150}))
                    best = None
                    for name, costs in cand:
                        trial = dict(tot)
                        for k, v in costs.items():
                            trial[k] += v
                        m = max(trial.values())
                        if best is None or m < best[0]:
                            best = (m, name, trial)
                    path[(t, i, s0)] = best[1]
                    tot = best[2]
    return path

PATH = _assign_paths()

# w SBUF/DRAM packing: per chunk, even fields pack into partitions 0-63,
# odd fields into 64-127, each group's pair-columns concatenated from col 0.
# Chunk width = max of the two groups' widths.
W_OFF = {}        # field -> col offset inside its chunk tile
W_CHUNK_W = []    # chunk -> tile width (cols)
for ch in CHUNKS:
    off = [0, 0]
    for i in ch:
        W_OFF[i] = off[i % 2]
        off[i % 2] += N_PAIRS[i] * D
    W_CHUNK_W.append(max(off))
W_COLS = sum(W_CHUNK_W)
W_CSTART = [sum(W_CHUNK_W[:ci]) for ci in range(len(CHUNKS))]

F32 = mybir.dt.float32
BF16 = mybir.dt.bfloat16


def build_bass() -> bass.Bass:
    # Bacc (not Bass): its compile() splits multi-sem waits into event
    # semaphores - TRN2 engine instructions take at most one inline wait.
    nc = bacc.Bacc()
    xn = nc.declare_dram_parameter("xn", [BC, F * D], BF16, isOutput=False)
    xt = nc.declare_dram_parameter("xt", [2 * D, NT * NSLOT * 128], BF16,
                                   isOutput=False)
    w = nc.declare_dram_parameter("w", [2 * D, W_COLS], BF16, isOutput=False)
    out = nc.declare_dram_parameter("out", [BC, OUT_COLS], BF16, isOutput=True)

    with TileContext(nc) as tc:
        with (
            tc.tile_pool(name="consts", bufs=1) as consts,
            tc.tile_pool(name="xn_pool", bufs=4) as xn_pool,
            tc.tile_pool(name="stage", bufs=7) as stage_pool,
            tc.tile_pool(name="cp_pool", bufs=4) as cp_pool,
            tc.tile_pool(name="psum", bufs=4, space="PSUM") as psum_pool,
        ):
            # PE warm-up: zeroed operands, results never read. Emitted first
            # so the matmul queue streams during the input-load phase.
            warm_w = consts.tile([D, 128], BF16, tag="warm_w", name="warm_w")
            warm_x = consts.tile([D, 512], BF16, tag="warm_x", name="warm_x")
            nc.vector.memset(warm_w[:], 0.0)
            nc.vector.memset(warm_x[:], 0.0)
            for _ in range(N_WARM):
                wps = psum_pool.tile([128, 512], F32, tag="ps")
                nc.tensor.matmul(wps[:], warm_w[:], warm_x[:],
                                 start=True, stop=True)

            w_sb = [consts.tile([2 * D, cw], BF16, tag=f"w{ci}", name=f"w{ci}")
                    for ci, cw in enumerate(W_CHUNK_W)]
            xt_sb = [consts.tile([2 * D, NSLOT * 128], BF16, tag=f"xt{t}",
                                 name=f"xtsb{t}")
                     for t in range(NT)]
            xn_sb = [xn_pool.tile([128, F * D], BF16, tag="xn", name=f"xn{t}")
                     for t in range(NT)]

            # Input loads, issued up front and ordered by first use. The
            # first ~1.7MB rides the HWDGE (nc.sync) ring: SWDGE descriptor
            # generation is dead until the GPSIMD ucode load finishes (~6us
            # of MODIFY_POOL_CONFIG at kernel start), while HWDGE needs no
            # Q7 software and streams from t~0. The rest rides SWDGE
            # (nc.gpsimd) so it never queues behind the output write stream
            # (HWDGE is strict FIFO per ring).
            def _load_w(ci, eng):
                eng.dma_start(
                    out=w_sb[ci][:],
                    in_=w[:, W_CSTART[ci]:W_CSTART[ci] + W_CHUNK_W[ci]])

            def _load_t(t, eng):
                s = t * NSLOT * 128
                eng.dma_start(out=xt_sb[t][:], in_=xt[:, s:s + NSLOT * 128])
                eng.dma_start(out=xn_sb[t][:],
                              in_=xn[t * 128:(t + 1) * 128, :])

            _load_w(0, nc.gpsimd); _load_t(0, nc.gpsimd)
            _load_w(1, nc.gpsimd); _load_w(2, nc.gpsimd)
            _load_t(1, nc.gpsimd); _load_w(3, nc.gpsimd); _load_w(4, nc.gpsimd)
            _load_t(2, nc.gpsimd); _load_w(5, nc.gpsimd); _load_w(6, nc.gpsimd)
            _load_t(3, nc.gpsimd)

            for t in range(NT):
                for ci, ch in enumerate(CHUNKS):
                    ccol0 = P_START[ch[0]] * D
                    ccol1 = (P_START[ch[-1]] + N_PAIRS[ch[-1]]) * D
                    ccols = ccol1 - ccol0
                    st = stage_pool.tile([128, ccols], BF16, tag="stage")
                    for i in ch:
                        npair = N_PAIRS[i]
                        cols = npair * D
                        wcol0 = P_START[i] * D
                        r0 = (i % 2) * D  # PE row group alternates per field
                        slot = i // 2
                        lhsT = xt_sb[t][r0:r0 + D, slot * 128:(slot + 1) * 128]
                        woff = W_OFF[i]
                        for s0, pc in _pieces(cols):
                            ps = psum_pool.tile([128, pc], F32, tag="ps")
                            for q0 in range(0, pc, 512):
                                n = min(512, pc - q0)
                                c0 = woff + s0 + q0
                                nc.tensor.matmul(
                                    ps[:, q0:q0 + n], lhsT,
                                    w_sb[ci][r0:r0 + D, c0:c0 + n],
                                    start=True, stop=True,
                                )
                            d0 = wcol0 - ccol0 + s0
                            dst = st[:, d0:d0 + pc]
                            xj = xn_sb[t][:, (i + 1) * D + s0:
                                          (i + 1) * D + s0 + pc]
                            p = PATH[(t, i, s0)]
                            if p in ("A", "B"):
                                cp = cp_pool.tile([128, pc], BF16, tag="cp")
                                nc.scalar.copy(out=cp[:], in_=ps[:])
                                eng = nc.vector if p == "A" else nc.gpsimd
                                eng.tensor_mul(dst, cp[:], xj)
                            else:
                                nc.vector.tensor_mul(dst, ps[:], xj)
                    nc.sync.dma_start(
                        out=out[t * 128:(t + 1) * 128, ccol0:ccol0 + ccols],
                        in_=st[:],
                    )
    nc.compile()
    return nc


def prep_inputs(x: np.ndarray, W: np.ndarray):
    """Full inputs -> per-core in_maps with pre-packed layouts."""
    x = np.ascontiguousarray(np.asarray(x, dtype=np.float32))
    W = np.ascontiguousarray(np.asarray(W, dtype=np.float32))
    # w: [128, W_COLS] bf16; chunk ci at cols W_CSTART[ci]; field i in rows
    # (i%2)*64..+64 at chunk-local col W_OFF[i]; identical on every core.
    wg = np.zeros((2 * D, W_COLS), dtype=ml_dtypes.bfloat16)
    wt = W.transpose(1, 0, 2)  # [D, P, D]
    for ci, ch in enumerate(CHUNKS):
        for i in ch:
            r0 = (i % 2) * D
            c0 = W_CSTART[ci] + W_OFF[i]
            cols = N_PAIRS[i] * D
            wg[r0:r0 + D, c0:c0 + cols] = (
                wt[:, P_START[i]:P_START[i] + N_PAIRS[i], :]
                .reshape(D, cols).astype(ml_dtypes.bfloat16)
            )
    in_maps = []
    for c in range(NCORES):
        xc = x[:, c * BC:(c + 1) * BC, :]                      # [26, 512, 64]
        xn = np.ascontiguousarray(
            xc.transpose(1, 0, 2).reshape(BC, F * D).astype(ml_dtypes.bfloat16)
        )
        # xt: [128, NT*13*128]; tile t, field i -> rows (i%2)*64..+64,
        # cols (t*13 + i//2)*128..+128, content xc[i, trows, :].T
        xt = np.zeros((2 * D, NT * NSLOT * 128), dtype=ml_dtypes.bfloat16)
        xct = (xc.reshape(F, NT, 128, D).transpose(0, 1, 3, 2)
               .astype(ml_dtypes.bfloat16))                    # [F, NT, D, 128]
        for i in range(F):
            r0 = (i % 2) * D
            for t in range(NT):
                c0 = (t * NSLOT + i // 2) * 128
                xt[r0:r0 + D, c0:c0 + 128] = xct[i, t]
        in_maps.append({"xn": xn, "xt": np.ascontiguousarray(xt), "w": wg})
    return in_maps


_CACHED_NC = None


def kernel(x: np.ndarray, W: np.ndarray) -> np.ndarray:
    global _CACHED_NC
    from concourse.bass_utils import run_bass_kernel_spmd

    if _CACHED_NC is None:
        _CACHED_NC = build_bass()
    in_maps = prep_inputs(x, W)
    res = run_bass_kernel_spmd(_CACHED_NC, in_maps, list(range(NCORES)))
    shards = [
        np.asarray(res.results[c]["out"]).astype(np.float32) for c in range(NCORES)
    ]
    return np.concatenate(shards, axis=0)
